# revision 43
# baseline (speedup 1.0000x reference)
"""AGCRN (adaptive graph conv GRU) Trainium2 kernel.

Model (B=64, L=24, N=512, F=2, H=128, ED=16, HOR=12):
  A = softmax(relu((E@W1)(E@W2)^T))                       [N,N]
  scan over L:  inp=[x_t, h];  g=A@inp;  z=sig(g@Wz+bz); r=sig(g@Wr+br)
                gh=A@[x_t, r*h]; ht=tanh(gh@Wh+bh); h=(1-z)h+z*ht
  out = (h@Whead + bhead) transposed to [B, HOR, N]

Distribution: pure data-parallel over batch B across 8 NeuronCores
(8 batches/core), weights + A replicated, no collectives.

Per-core layouts (node-major = node index on SBUF partitions):
  AT_c   [128, 512] f32r  c=0..3   : A^T chunks, conv moving operand
  AXT_b  [48, 512]  f32r  b=0..7   : (A @ x_t)^T for all t, rows (t,f)
  hT     [128, 8*512] f32r         : h^T, partitions=H, free=(b,n)
  hN     [128, 4, 8, 128] f32r     : h node-major, free=(chunk,b,H)
Step math (per batch b):
  G^T  = lhsT(hN[:,c,b,:]) x rhs(AT_c) accum over c      -> psum [H,512]
  zpre = lhsT(Wz[0:2]) x AXT_b[2t:2t+2] + lhsT(Wz[2:]) x G^T(sbuf)
  z    = ACT sigmoid(zpre + bz);  r likewise
  rh   = gpsimd r*hT ; rhN = PE-transpose(rh) ; Gh^T like G^T
  ht   = ACT tanh(...); hT_new = hT + z*(ht - hT) on DVE
  hN_new = PE-transpose(hT_new)
"""
import numpy as np
from contextlib import ExitStack as _ExitStack

import concourse.bass as bass
import concourse.mybir as mybir
import concourse.tile as tile
from concourse import bacc
from concourse.bass_utils import run_bass_kernel_spmd
from concourse.masks import make_identity

F32 = mybir.dt.float32
F32R = mybir.dt.float32r
BF16 = mybir.dt.bfloat16
AF = mybir.ActivationFunctionType
ALU = mybir.AluOpType

B, L, N, F, H, ED, HOR = 64, 24, 512, 2, 128, 16, 12
NCORES = 8
NB = B // NCORES          # batches per core
NC4 = N // 128            # node chunks


def build_nc():
    nc = bacc.Bacc("TRN2", target_bir_lowering=False, debug=False,
                   num_devices=NCORES)

    xn_ext = nc.declare_dram_parameter("Xn", [N, NB, 64], F32, isOutput=False)
    wxp_ext = nc.declare_dram_parameter("WxP", [3, 16, 32, H], F32, isOutput=False)
    et_ext = nc.declare_dram_parameter("ET", [ED, N], F32, isOutput=False)
    w1_ext = nc.declare_dram_parameter("W1", [ED, ED], F32, isOutput=False)
    w2_ext = nc.declare_dram_parameter("W2", [ED, ED], F32, isOutput=False)
    wz_ext = nc.declare_dram_parameter("Wz", [F + H, H], F32, isOutput=False)
    wr_ext = nc.declare_dram_parameter("Wr", [F + H, H], F32, isOutput=False)
    wh_ext = nc.declare_dram_parameter("Wh", [F + H, H], F32, isOutput=False)
    bz_ext = nc.declare_dram_parameter("bz", [H], F32, isOutput=False)
    br_ext = nc.declare_dram_parameter("br", [H], F32, isOutput=False)
    bh_ext = nc.declare_dram_parameter("bh", [H], F32, isOutput=False)
    whd_ext = nc.declare_dram_parameter("Whead", [H, HOR], F32, isOutput=False)
    bhd_ext = nc.declare_dram_parameter("bhead", [HOR], F32, isOutput=False)
    out_ext = nc.declare_dram_parameter("out", [NB, HOR, N], F32, isOutput=True)

    with tile.TileContext(nc) as tc:
        with tc.tile_pool(name="const", bufs=1) as cpool:

            # ---- constants ----
            ident = cpool.tile([128, 128], F32, tag="ident")
            make_identity(nc, ident[:])
            ident_r = cpool.tile([128, 128], F32R, tag="identr")
            nc.vector.tensor_copy(ident_r[:], ident[:])
            ident_b = cpool.tile([128, 128], BF16, tag="identb")
            nc.vector.tensor_copy(ident_b[:], ident[:])

            wzh = cpool.tile([H, H], BF16, tag="wzh")
            wrh = cpool.tile([H, H], BF16, tag="wrh")
            whh = cpool.tile([H, H], BF16, tag="whh")
            for wext, wh_ in ((wz_ext, wzh), (wr_ext, wrh), (wh_ext, whh)):
                nc.gpsimd.dma_start(wh_[:], wext[F:F + H, :])
            # zero-padded k=32 stationaries for the x-side of each gate:
            # wxp[:, g, s, :] has Wg[0:2] at rows (2s, 2s+1), zeros elsewhere
            wxp = cpool.tile([32, 3, 16, H], BF16, tag="wxp")
            nc.gpsimd.dma_start(wxp[:], wxp_ext[:].transpose([2, 0, 1, 3]))
            bz_sb = cpool.tile([H, 1], F32, tag="bz")
            br_sb = cpool.tile([H, 1], F32, tag="br")
            bh_sb = cpool.tile([H, 1], F32, tag="bh")
            nc.sync.dma_start(bz_sb[:], bz_ext[:].unsqueeze(-1))
            nc.sync.dma_start(br_sb[:], br_ext[:].unsqueeze(-1))
            nc.sync.dma_start(bh_sb[:], bh_ext[:].unsqueeze(-1))
            whd_sb = cpool.tile([H, HOR], BF16, tag="whd")
            nc.gpsimd.dma_start(whd_sb[:], whd_ext[:])
            bhd_sb = cpool.tile([HOR, 1], F32, tag="bhd")
            nc.sync.dma_start(bhd_sb[:], bhd_ext[:].unsqueeze(-1))

            AT = [cpool.tile([128, N], BF16, tag=f"AT{c}", name=f"AT{c}") for c in range(NC4)]
            AXT = [cpool.tile([32, 2, N], BF16, tag=f"AXT{b}", name=f"AXT{b}") for b in range(NB)]

            # ---- adjacency precompute ----
            with tc.tile_pool(name="pre", bufs=1) as pre, \
                 tc.tile_pool(name="ppre", bufs=2, space="PSUM") as ppre:
                et_sb = pre.tile([ED, N], F32R, tag="et")
                w1_sb = pre.tile([ED, ED], F32R, tag="w1")
                w2_sb = pre.tile([ED, ED], F32R, tag="w2")
                nc.sync.dma_start(et_sb[:], et_ext[:].bitcast(F32R))
                nc.sync.dma_start(w1_sb[:], w1_ext[:].bitcast(F32R))
                nc.sync.dma_start(w2_sb[:], w2_ext[:].bitcast(F32R))

                m1t = pre.tile([ED, N], F32R, tag="m1t")
                m2t = pre.tile([ED, N], F32R, tag="m2t")
                for wsb, mt in ((w1_sb, m1t), (w2_sb, m2t)):
                    ps = ppre.tile([ED, N], F32, tag="mps")
                    nc.tensor.matmul(ps[:], wsb[:], et_sb[:], start=True, stop=True)
                    nc.scalar.copy(mt[:], ps[:])

                A_sb = [pre.tile([128, N], F32, tag=f"A{i}", name=f"A{i}") for i in range(NC4)]
                for i in range(NC4):
                    ps = ppre.tile([128, N], F32, tag="sps")
                    nc.tensor.matmul(ps[:], m1t[:, i * 128:(i + 1) * 128],
                                     m2t[:], start=True, stop=True)
                    s_sb = pre.tile([128, N], F32, tag="s")
                    nc.scalar.activation(s_sb[:], ps[:], AF.Relu)
                    mx = pre.tile([128, 1], F32, tag="mx")
                    nc.vector.tensor_reduce(mx[:], s_sb[:],
                                            axis=mybir.AxisListType.X, op=ALU.max)
                    nmx = pre.tile([128, 1], F32, tag="nmx")
                    nc.vector.tensor_scalar_mul(nmx[:], mx[:], -1.0)
                    sm = pre.tile([128, 1], F32, tag="sm")
                    nc.scalar.activation(A_sb[i][:], s_sb[:], AF.Exp,
                                         bias=nmx[:], accum_out=sm[:])
                    rs = pre.tile([128, 1], F32, tag="rs")
                    nc.vector.reciprocal(rs[:], sm[:])
                    nc.vector.tensor_scalar_mul(A_sb[i][:], A_sb[i][:], rs[:])

                # AT = A^T via 16 PE transposes
                for c in range(NC4):
                    for i in range(NC4):
                        tp = ppre.tile([128, 128], F32, tag="tp")
                        nc.tensor.transpose(tp[:],
                                            A_sb[i][:, c * 128:(c + 1) * 128],
                                            ident[:])
                        nc.vector.tensor_copy(AT[c][:, i * 128:(i + 1) * 128],
                                              tp[:])

                # AXT_b = (A @ x_t)^T for all t: lhsT = Xn chunk [(128, (t,f)]
                xn_sb = [pre.tile([128, NB, 64], BF16, tag=f"xn{c}", name=f"xn{c}")
                         for c in range(NC4)]
                for c in range(NC4):
                    nc.gpsimd.dma_start(
                        xn_sb[c][:], xn_ext[c * 128:(c + 1) * 128])
                for b in range(NB):
                    for g in range(2):
                        ps = ppre.tile([32, N], F32, tag="axps")
                        for c in range(NC4):
                            nc.tensor.matmul(ps[:],
                                             xn_sb[c][:, b, 32 * g:32 * (g + 1)],
                                             AT[c][:], start=(c == 0),
                                             stop=(c == NC4 - 1))
                        nc.vector.tensor_copy(AXT[b][:, g, :], ps[:])

            # ---- scan over L ----
            _scan_stack = _ExitStack()
            spool = _scan_stack.enter_context(tc.tile_pool(name="state", bufs=2))
            wpool = _scan_stack.enter_context(tc.tile_pool(name="work", bufs=6))
            pgate = _scan_stack.enter_context(
                tc.tile_pool(name="pgate", bufs=4, space="PSUM"))
            pconv = _scan_stack.enter_context(
                tc.tile_pool(name="pconv", bufs=2, space="PSUM"))
            ptr = _scan_stack.enter_context(
                tc.tile_pool(name="ptr", bufs=2, space="PSUM"))
            hT_prev = [None] * NB
            hN_prev = [None] * NB
            GS = 1
            for t in range(L):
                g_idx, s_idx = t // 16, t % 16
                for grp in range(0, NB, GS):
                    bl = range(grp, min(grp + GS, NB))
                    axt = {b: AXT[b][:, g_idx, :] for b in bl}
                    hT_new = {b: spool.tile([H, N], BF16, tag=f"hT{b}",
                                            name=f"hT{b}_{t}") for b in bl}
                    hN_new = {b: spool.tile([128, NC4, H], BF16, tag=f"hN{b}",
                                            name=f"hN{b}_{t}") for b in bl}
                    g_sb, z_sb, r_sb, rh_sb, rhn_sb, gh_sb, ht_sb = (
                        {}, {}, {}, {}, {}, {}, {})

                    if t > 0:
                        # phase A: conv1; ACT copies chase
                        for b in bl:
                            gps = pconv.tile([H, N], F32, tag="gps",
                                             name=f"gps{t}_{b}")
                            for c in range(NC4):
                                nc.tensor.matmul(gps[:], hN_prev[b][:, c, :],
                                                 AT[c][:], start=(c == 0),
                                                 stop=(c == NC4 - 1))
                            g_sb[b] = wpool.tile([H, N], BF16, tag="G",
                                                 name=f"G{t}_{b}")
                            nc.scalar.copy(g_sb[b][:], gps[:])
                        # phase B: r gates for the whole group first
                        rps_l, zps_l = {}, {}
                        for b in bl:
                            rps_l[b] = pgate.tile([H, N], F32, tag="gate",
                                                  name=f"rps{t}_{b}")
                            nc.tensor.matmul(rps_l[b][:], wxp[:, 1, s_idx, :],
                                             axt[b], start=True, stop=False)
                            nc.tensor.matmul(rps_l[b][:], wrh[:], g_sb[b][:],
                                             start=False, stop=True)
                        for b in bl:
                            r_sb[b] = wpool.tile([H, N], BF16, tag="r",
                                                 name=f"r{t}_{b}")
                            nc.scalar.activation(r_sb[b][:], rps_l[b][:],
                                                 AF.Sigmoid, bias=br_sb[:])
                            zps_l[b] = pgate.tile([H, N], F32, tag="gate",
                                                  name=f"zps{t}_{b}")
                            nc.tensor.matmul(zps_l[b][:], wxp[:, 0, s_idx, :],
                                             axt[b], start=True, stop=False)
                            nc.tensor.matmul(zps_l[b][:], wzh[:], g_sb[b][:],
                                             start=False, stop=True)
                        for b in bl:
                            z_sb[b] = wpool.tile([H, N], BF16, tag="z",
                                                 name=f"z{t}_{b}")
                            nc.scalar.activation(z_sb[b][:], zps_l[b][:],
                                                 AF.Sigmoid, bias=bz_sb[:])
                        # phase C: transpose r, then rhN = rT (x) hN fused
                        # on DVE (psum operand + elementwise mul in one op)
                        for b in bl:
                            rtp = ptr.tile([128, N], BF16, tag="tps",
                                           name=f"rtp{t}_{b}")
                            for j in range(NC4):
                                nc.tensor.transpose(
                                    rtp[:, j * 128:(j + 1) * 128],
                                    r_sb[b][:, j * 128:(j + 1) * 128],
                                    ident_b[:])
                            rhn_sb[b] = wpool.tile([128, N], BF16, tag="rhN",
                                                   name=f"rhN{t}_{b}")
                            nc.vector.tensor_mul(
                                rhn_sb[b][:], rtp[:],
                                hN_prev[b][:].rearrange("p c h -> p (c h)"))
                        # phase D: conv2
                        for b in bl:
                            ghps = pconv.tile([H, N], F32, tag="gps",
                                              name=f"ghps{t}_{b}")
                            for c in range(NC4):
                                nc.tensor.matmul(
                                    ghps[:],
                                    rhn_sb[b][:, c * 128:(c + 1) * 128],
                                    AT[c][:], start=(c == 0),
                                    stop=(c == NC4 - 1))
                            gh_sb[b] = wpool.tile([H, N], BF16, tag="Gh",
                                                  name=f"Gh{t}_{b}")
                            nc.scalar.copy(gh_sb[b][:], ghps[:])

                    # phase E+F: candidate gate, tanh, GRU update
                    for b in bl:
                        hps = pgate.tile([H, N], F32, tag="gate",
                                         name=f"hps{t}_{b}")
                        nc.tensor.matmul(hps[:], wxp[:, 2, s_idx, :], axt[b],
                                         start=True, stop=(t == 0))
                        if t > 0:
                            nc.tensor.matmul(hps[:], whh[:], gh_sb[b][:],
                                             start=False, stop=True)
                        ht_sb[b] = wpool.tile([H, N], BF16, tag="ht",
                                              name=f"ht{t}_{b}")
                        nc.scalar.activation(ht_sb[b][:], hps[:], AF.Tanh,
                                             bias=bh_sb[:])
                        if t == 0:
                            zps = pgate.tile([H, N], F32, tag="gate",
                                             name=f"zps{t}_{b}")
                            nc.tensor.matmul(zps[:], wxp[:, 0, s_idx, :],
                                             axt[b], start=True, stop=True)
                            z_sb[b] = wpool.tile([H, N], BF16, tag="z",
                                                 name=f"z{t}_{b}")
                            nc.scalar.activation(z_sb[b][:], zps[:],
                                                 AF.Sigmoid, bias=bz_sb[:])
                            nc.vector.tensor_mul(hT_new[b][:], z_sb[b][:],
                                                 ht_sb[b][:])
                        else:
                            d_sb = wpool.tile([H, N], BF16, tag="d",
                                              name=f"d{t}_{b}")
                            nc.vector.tensor_sub(d_sb[:], ht_sb[b][:],
                                                 hT_prev[b][:])
                            nc.vector.tensor_mul(d_sb[:], z_sb[b][:], d_sb[:])
                            nc.vector.tensor_add(hT_new[b][:],
                                                 hT_prev[b][:], d_sb[:])
                    # phase G: transpose new h to node-major (skip at t=L-1)
                    for b in (bl if t < L - 1 else []):
                        htp = ptr.tile([128, N], BF16, tag="tps",
                                       name=f"htp{t}_{b}")
                        for j in range(NC4):
                            nc.tensor.transpose(
                                htp[:, j * 128:(j + 1) * 128],
                                hT_new[b][:, j * 128:(j + 1) * 128],
                                ident_b[:])
                        nc.vector.tensor_copy(
                            hN_new[b][:].rearrange("p c h -> p (c h)"),
                            htp[:])
                    for b in bl:
                        hT_prev[b] = hT_new[b]
                        hN_prev[b] = hN_new[b]

            # ---- head ----
            for b in range(NB):
                hd = pgate.tile([HOR, N], F32, tag="gate")
                nc.tensor.matmul(hd[:], whd_sb[:], hT_prev[b][:],
                                 start=True, stop=True)
                o_sb = wpool.tile([HOR, N], F32, tag="o")
                nc.scalar.activation(o_sb[:], hd[:], AF.Identity, bias=bhd_sb[:])
                nc.sync.dma_start(out_ext[b], o_sb[:])
            _scan_stack.close()

    nc.compile()
    return nc


_NC_CACHE = None


def _get_nc():
    global _NC_CACHE
    if _NC_CACHE is None:
        _NC_CACHE = build_nc()
    return _NC_CACHE


def make_in_maps(X, E, W1, W2, Wz, bz, Wr, br, Wh, bh, Whead, bhead):
    ET = np.ascontiguousarray(np.asarray(E, dtype=np.float32).T)
    common = {
        "ET": ET, "W1": np.asarray(W1, np.float32), "W2": np.asarray(W2, np.float32),
        "Wz": np.asarray(Wz, np.float32), "Wr": np.asarray(Wr, np.float32),
        "Wh": np.asarray(Wh, np.float32),
        "bz": np.asarray(bz, np.float32), "br": np.asarray(br, np.float32),
        "bh": np.asarray(bh, np.float32),
        "Whead": np.asarray(Whead, np.float32),
        "bhead": np.asarray(bhead, np.float32),
    }
    wxp = np.zeros((3, 16, 32, H), np.float32)
    for gi, W in enumerate((Wz, Wr, Wh)):
        Wx = np.asarray(W, np.float32)[0:F, :]
        for si in range(16):
            wxp[gi, si, 2 * si:2 * si + 2, :] = Wx
    common["WxP"] = wxp
    X = np.asarray(X, np.float32)
    in_maps = []
    for i in range(NCORES):
        xs = X[i * NB:(i + 1) * NB]                       # [NB, L, N, F]
        xn = np.zeros((N, NB, 64), np.float32)
        xn[:, :, :L * F] = xs.transpose(2, 0, 1, 3).reshape(N, NB, L * F)
        in_maps.append({"Xn": xn, **common})
    return in_maps


def run_on_hw(inputs, **kwargs):
    nc = _get_nc()
    in_maps = make_in_maps(**inputs)
    last_err = None
    for _attempt in range(3):
        try:
            res = run_bass_kernel_spmd(nc, in_maps,
                                       core_ids=list(range(NCORES)), **kwargs)
            break
        except Exception as e:  # transient NRT_EXEC_UNIT_UNRECOVERABLE
            last_err = e
            if "UNRECOVERABLE" not in str(e) and "UNAVAILABLE" not in str(e):
                raise
    else:
        raise last_err
    out = np.concatenate([res.results[i]["out"] for i in range(NCORES)], axis=0)
    return out, res


def kernel(**inputs) -> np.ndarray:
    out, _ = run_on_hw(inputs)
    return out


# revision 44
# speedup vs baseline: 1.0093x; 1.0093x over previous
"""AGCRN (adaptive graph conv GRU) Trainium2 kernel.

Model (B=64, L=24, N=512, F=2, H=128, ED=16, HOR=12):
  A = softmax(relu((E@W1)(E@W2)^T))                       [N,N]
  scan over L:  inp=[x_t, h];  g=A@inp;  z=sig(g@Wz+bz); r=sig(g@Wr+br)
                gh=A@[x_t, r*h]; ht=tanh(gh@Wh+bh); h=(1-z)h+z*ht
  out = (h@Whead + bhead) transposed to [B, HOR, N]

Distribution: pure data-parallel over batch B across 8 NeuronCores
(8 batches/core), weights + A replicated, no collectives.

Per-core layouts (node-major = node index on SBUF partitions):
  AT_c   [128, 512] f32r  c=0..3   : A^T chunks, conv moving operand
  AXT_b  [48, 512]  f32r  b=0..7   : (A @ x_t)^T for all t, rows (t,f)
  hT     [128, 8*512] f32r         : h^T, partitions=H, free=(b,n)
  hN     [128, 4, 8, 128] f32r     : h node-major, free=(chunk,b,H)
Step math (per batch b):
  G^T  = lhsT(hN[:,c,b,:]) x rhs(AT_c) accum over c      -> psum [H,512]
  zpre = lhsT(Wz[0:2]) x AXT_b[2t:2t+2] + lhsT(Wz[2:]) x G^T(sbuf)
  z    = ACT sigmoid(zpre + bz);  r likewise
  rh   = gpsimd r*hT ; rhN = PE-transpose(rh) ; Gh^T like G^T
  ht   = ACT tanh(...); hT_new = hT + z*(ht - hT) on DVE
  hN_new = PE-transpose(hT_new)
"""
import numpy as np
from contextlib import ExitStack as _ExitStack

import concourse.bass as bass
import concourse.mybir as mybir
import concourse.tile as tile
from concourse import bacc
from concourse.bass_utils import run_bass_kernel_spmd
from concourse.masks import make_identity

F32 = mybir.dt.float32
F32R = mybir.dt.float32r
BF16 = mybir.dt.bfloat16
AF = mybir.ActivationFunctionType
ALU = mybir.AluOpType

B, L, N, F, H, ED, HOR = 64, 24, 512, 2, 128, 16, 12
NCORES = 8
NB = B // NCORES          # batches per core
NC4 = N // 128            # node chunks


def build_nc():
    nc = bacc.Bacc("TRN2", target_bir_lowering=False, debug=False,
                   num_devices=NCORES)

    xn_ext = nc.declare_dram_parameter("Xn", [N, NB, 64], F32, isOutput=False)
    wxp_ext = nc.declare_dram_parameter("WxP", [3, 16, 32, H], F32, isOutput=False)
    et_ext = nc.declare_dram_parameter("ET", [ED, N], F32, isOutput=False)
    w1_ext = nc.declare_dram_parameter("W1", [ED, ED], F32, isOutput=False)
    w2_ext = nc.declare_dram_parameter("W2", [ED, ED], F32, isOutput=False)
    wz_ext = nc.declare_dram_parameter("Wz", [F + H, H], F32, isOutput=False)
    wr_ext = nc.declare_dram_parameter("Wr", [F + H, H], F32, isOutput=False)
    wh_ext = nc.declare_dram_parameter("Wh", [F + H, H], F32, isOutput=False)
    bz_ext = nc.declare_dram_parameter("bz", [H], F32, isOutput=False)
    br_ext = nc.declare_dram_parameter("br", [H], F32, isOutput=False)
    bh_ext = nc.declare_dram_parameter("bh", [H], F32, isOutput=False)
    whd_ext = nc.declare_dram_parameter("Whead", [H, HOR], F32, isOutput=False)
    bhd_ext = nc.declare_dram_parameter("bhead", [HOR], F32, isOutput=False)
    out_ext = nc.declare_dram_parameter("out", [NB, HOR, N], F32, isOutput=True)

    with tile.TileContext(nc) as tc:
        with tc.tile_pool(name="const", bufs=1) as cpool:

            # ---- constants ----
            ident = cpool.tile([128, 128], F32, tag="ident")
            make_identity(nc, ident[:])
            ident_r = cpool.tile([128, 128], F32R, tag="identr")
            nc.vector.tensor_copy(ident_r[:], ident[:])
            ident_b = cpool.tile([128, 128], BF16, tag="identb")
            nc.vector.tensor_copy(ident_b[:], ident[:])

            wzh = cpool.tile([H, H], BF16, tag="wzh")
            wrh = cpool.tile([H, H], BF16, tag="wrh")
            whh = cpool.tile([H, H], BF16, tag="whh")
            for wext, wh_ in ((wz_ext, wzh), (wr_ext, wrh), (wh_ext, whh)):
                nc.gpsimd.dma_start(wh_[:], wext[F:F + H, :])
            # zero-padded k=32 stationaries for the x-side of each gate:
            # wxp[:, g, s, :] has Wg[0:2] at rows (2s, 2s+1), zeros elsewhere
            wxp = cpool.tile([32, 3, 16, H], BF16, tag="wxp")
            nc.gpsimd.dma_start(wxp[:], wxp_ext[:].transpose([2, 0, 1, 3]))
            bz_sb = cpool.tile([H, 1], F32, tag="bz")
            br_sb = cpool.tile([H, 1], F32, tag="br")
            bh_sb = cpool.tile([H, 1], F32, tag="bh")
            nc.sync.dma_start(bz_sb[:], bz_ext[:].unsqueeze(-1))
            nc.sync.dma_start(br_sb[:], br_ext[:].unsqueeze(-1))
            nc.sync.dma_start(bh_sb[:], bh_ext[:].unsqueeze(-1))
            whd_sb = cpool.tile([H, HOR], BF16, tag="whd")
            nc.gpsimd.dma_start(whd_sb[:], whd_ext[:])
            bhd_sb = cpool.tile([HOR, 1], F32, tag="bhd")
            nc.sync.dma_start(bhd_sb[:], bhd_ext[:].unsqueeze(-1))

            AT = [cpool.tile([128, N], BF16, tag=f"AT{c}", name=f"AT{c}") for c in range(NC4)]
            AXT = [cpool.tile([32, 2, N], BF16, tag=f"AXT{b}", name=f"AXT{b}") for b in range(NB)]

            # ---- adjacency precompute ----
            with tc.tile_pool(name="pre", bufs=1) as pre, \
                 tc.tile_pool(name="ppre", bufs=2, space="PSUM") as ppre:
                et_sb = pre.tile([ED, N], F32R, tag="et")
                w1_sb = pre.tile([ED, ED], F32R, tag="w1")
                w2_sb = pre.tile([ED, ED], F32R, tag="w2")
                nc.sync.dma_start(et_sb[:], et_ext[:].bitcast(F32R))
                nc.sync.dma_start(w1_sb[:], w1_ext[:].bitcast(F32R))
                nc.sync.dma_start(w2_sb[:], w2_ext[:].bitcast(F32R))

                m1t = pre.tile([ED, N], F32R, tag="m1t")
                m2t = pre.tile([ED, N], F32R, tag="m2t")
                for wsb, mt in ((w1_sb, m1t), (w2_sb, m2t)):
                    ps = ppre.tile([ED, N], F32, tag="mps")
                    nc.tensor.matmul(ps[:], wsb[:], et_sb[:], start=True, stop=True)
                    nc.scalar.copy(mt[:], ps[:])

                A_sb = [pre.tile([128, N], F32, tag=f"A{i}", name=f"A{i}") for i in range(NC4)]
                for i in range(NC4):
                    ps = ppre.tile([128, N], F32, tag="sps")
                    nc.tensor.matmul(ps[:], m1t[:, i * 128:(i + 1) * 128],
                                     m2t[:], start=True, stop=True)
                    s_sb = pre.tile([128, N], F32, tag="s")
                    nc.scalar.activation(s_sb[:], ps[:], AF.Relu)
                    mx = pre.tile([128, 1], F32, tag="mx")
                    nc.vector.tensor_reduce(mx[:], s_sb[:],
                                            axis=mybir.AxisListType.X, op=ALU.max)
                    nmx = pre.tile([128, 1], F32, tag="nmx")
                    nc.vector.tensor_scalar_mul(nmx[:], mx[:], -1.0)
                    sm = pre.tile([128, 1], F32, tag="sm")
                    nc.scalar.activation(A_sb[i][:], s_sb[:], AF.Exp,
                                         bias=nmx[:], accum_out=sm[:])
                    rs = pre.tile([128, 1], F32, tag="rs")
                    nc.vector.reciprocal(rs[:], sm[:])
                    nc.vector.tensor_scalar_mul(A_sb[i][:], A_sb[i][:], rs[:])

                # AT = A^T via 16 PE transposes
                for c in range(NC4):
                    for i in range(NC4):
                        tp = ppre.tile([128, 128], F32, tag="tp")
                        nc.tensor.transpose(tp[:],
                                            A_sb[i][:, c * 128:(c + 1) * 128],
                                            ident[:])
                        nc.vector.tensor_copy(AT[c][:, i * 128:(i + 1) * 128],
                                              tp[:])

                # AXT_b = (A @ x_t)^T for all t: lhsT = Xn chunk [(128, (t,f)]
                xn_sb = [pre.tile([128, NB, 64], BF16, tag=f"xn{c}", name=f"xn{c}")
                         for c in range(NC4)]
                for c in range(NC4):
                    nc.gpsimd.dma_start(
                        xn_sb[c][:], xn_ext[c * 128:(c + 1) * 128])
                for b in range(NB):
                    for g in range(2):
                        ps = ppre.tile([32, N], F32, tag="axps")
                        for c in range(NC4):
                            nc.tensor.matmul(ps[:],
                                             xn_sb[c][:, b, 32 * g:32 * (g + 1)],
                                             AT[c][:], start=(c == 0),
                                             stop=(c == NC4 - 1))
                        nc.vector.tensor_copy(AXT[b][:, g, :], ps[:])

            # ---- scan over L ----
            _scan_stack = _ExitStack()
            spool = _scan_stack.enter_context(tc.tile_pool(name="state", bufs=2))
            wpool = _scan_stack.enter_context(tc.tile_pool(name="work", bufs=6))
            pgate = _scan_stack.enter_context(
                tc.tile_pool(name="pgate", bufs=4, space="PSUM"))
            pconv = _scan_stack.enter_context(
                tc.tile_pool(name="pconv", bufs=2, space="PSUM"))
            ptr = _scan_stack.enter_context(
                tc.tile_pool(name="ptr", bufs=2, space="PSUM"))
            hT_prev = [None] * NB
            hN_prev = [None] * NB
            GS = 1
            for t in range(L):
                g_idx, s_idx = t // 16, t % 16
                for grp in range(0, NB, GS):
                    bl = range(grp, min(grp + GS, NB))
                    axt = {b: AXT[b][:, g_idx, :] for b in bl}
                    hT_new = {b: spool.tile([H, N], BF16, tag=f"hT{b}",
                                            name=f"hT{b}_{t}") for b in bl}
                    hN_new = {b: spool.tile([128, NC4, H], BF16, tag=f"hN{b}",
                                            name=f"hN{b}_{t}") for b in bl}
                    g_sb, z_sb, r_sb, rh_sb, rhn_sb, gh_sb, ht_sb = (
                        {}, {}, {}, {}, {}, {}, {})

                    if t > 0:
                        # phase A: conv1; ACT copies chase
                        for b in bl:
                            gps = pconv.tile([H, N], F32, tag="gps",
                                             name=f"gps{t}_{b}")
                            for c in range(NC4):
                                nc.tensor.matmul(gps[:], hN_prev[b][:, c, :],
                                                 AT[c][:], start=(c == 0),
                                                 stop=(c == NC4 - 1))
                            g_sb[b] = wpool.tile([H, N], BF16, tag="G",
                                                 name=f"G{t}_{b}")
                            nc.scalar.copy(g_sb[b][:], gps[:])
                        # phase B: z and r gates
                        for b in bl:
                            rps = pgate.tile([H, N], F32, tag="gate",
                                             name=f"rps{t}_{b}")
                            nc.tensor.matmul(rps[:], wxp[:, 1, s_idx, :],
                                             axt[b], start=True, stop=False)
                            nc.tensor.matmul(rps[:], wrh[:], g_sb[b][:],
                                             start=False, stop=True)
                            zps = pgate.tile([H, N], F32, tag="gate",
                                             name=f"zps{t}_{b}")
                            nc.tensor.matmul(zps[:], wxp[:, 0, s_idx, :],
                                             axt[b], start=True, stop=False)
                            nc.tensor.matmul(zps[:], wzh[:], g_sb[b][:],
                                             start=False, stop=True)
                            r_sb[b] = wpool.tile([H, N], BF16, tag="r",
                                                 name=f"r{t}_{b}")
                            nc.scalar.activation(r_sb[b][:], rps[:],
                                                 AF.Sigmoid, bias=br_sb[:])
                            z_sb[b] = wpool.tile([H, N], BF16, tag="z",
                                                 name=f"z{t}_{b}")
                            nc.scalar.activation(z_sb[b][:], zps[:],
                                                 AF.Sigmoid, bias=bz_sb[:])
                        # phase C: transpose r, then rhN = rT (x) hN fused
                        # on DVE (psum operand + elementwise mul in one op)
                        for b in bl:
                            rtp = ptr.tile([128, N], BF16, tag="tps",
                                           name=f"rtp{t}_{b}")
                            for j in range(NC4):
                                nc.tensor.transpose(
                                    rtp[:, j * 128:(j + 1) * 128],
                                    r_sb[b][:, j * 128:(j + 1) * 128],
                                    ident_b[:])
                            rhn_sb[b] = wpool.tile([128, N], BF16, tag="rhN",
                                                   name=f"rhN{t}_{b}")
                            nc.vector.tensor_mul(
                                rhn_sb[b][:], rtp[:],
                                hN_prev[b][:].rearrange("p c h -> p (c h)"))
                        # phase D: conv2
                        for b in bl:
                            ghps = pconv.tile([H, N], F32, tag="gps",
                                              name=f"ghps{t}_{b}")
                            for c in range(NC4):
                                nc.tensor.matmul(
                                    ghps[:],
                                    rhn_sb[b][:, c * 128:(c + 1) * 128],
                                    AT[c][:], start=(c == 0),
                                    stop=(c == NC4 - 1))
                            gh_sb[b] = wpool.tile([H, N], BF16, tag="Gh",
                                                  name=f"Gh{t}_{b}")
                            nc.scalar.copy(gh_sb[b][:], ghps[:])

                    # phase E+F: candidate gate, tanh, GRU update
                    for b in bl:
                        hps = pgate.tile([H, N], F32, tag="gate",
                                         name=f"hps{t}_{b}")
                        nc.tensor.matmul(hps[:], wxp[:, 2, s_idx, :], axt[b],
                                         start=True, stop=(t == 0))
                        if t > 0:
                            nc.tensor.matmul(hps[:], whh[:], gh_sb[b][:],
                                             start=False, stop=True)
                        ht_sb[b] = wpool.tile([H, N], BF16, tag="ht",
                                              name=f"ht{t}_{b}")
                        nc.scalar.activation(ht_sb[b][:], hps[:], AF.Tanh,
                                             bias=bh_sb[:])
                        if t == 0:
                            zps = pgate.tile([H, N], F32, tag="gate",
                                             name=f"zps{t}_{b}")
                            nc.tensor.matmul(zps[:], wxp[:, 0, s_idx, :],
                                             axt[b], start=True, stop=True)
                            z_sb[b] = wpool.tile([H, N], BF16, tag="z",
                                                 name=f"z{t}_{b}")
                            nc.scalar.activation(z_sb[b][:], zps[:],
                                                 AF.Sigmoid, bias=bz_sb[:])
                            nc.vector.tensor_mul(hT_new[b][:], z_sb[b][:],
                                                 ht_sb[b][:])
                        else:
                            d_sb = wpool.tile([H, N], BF16, tag="d",
                                              name=f"d{t}_{b}")
                            nc.vector.tensor_sub(d_sb[:], ht_sb[b][:],
                                                 hT_prev[b][:])
                            nc.vector.tensor_mul(d_sb[:], z_sb[b][:], d_sb[:])
                            nc.vector.tensor_add(hT_new[b][:],
                                                 hT_prev[b][:], d_sb[:])
                    # phase G: transpose new h to node-major (skip at t=L-1)
                    for b in (bl if t < L - 1 else []):
                        htp = ptr.tile([128, N], BF16, tag="tps",
                                       name=f"htp{t}_{b}")
                        for j in range(NC4):
                            nc.tensor.transpose(
                                htp[:, j * 128:(j + 1) * 128],
                                hT_new[b][:, j * 128:(j + 1) * 128],
                                ident_b[:])
                        nc.vector.tensor_copy(
                            hN_new[b][:].rearrange("p c h -> p (c h)"),
                            htp[:])
                    for b in bl:
                        hT_prev[b] = hT_new[b]
                        hN_prev[b] = hN_new[b]

            # ---- head ----
            for b in range(NB):
                hd = pgate.tile([HOR, N], F32, tag="gate")
                nc.tensor.matmul(hd[:], whd_sb[:], hT_prev[b][:],
                                 start=True, stop=True)
                o_sb = wpool.tile([HOR, N], F32, tag="o")
                nc.scalar.activation(o_sb[:], hd[:], AF.Identity, bias=bhd_sb[:])
                nc.sync.dma_start(out_ext[b], o_sb[:])
            _scan_stack.close()

    nc.compile()
    return nc


_NC_CACHE = None


def _get_nc():
    global _NC_CACHE
    if _NC_CACHE is None:
        _NC_CACHE = build_nc()
    return _NC_CACHE


def make_in_maps(X, E, W1, W2, Wz, bz, Wr, br, Wh, bh, Whead, bhead):
    ET = np.ascontiguousarray(np.asarray(E, dtype=np.float32).T)
    common = {
        "ET": ET, "W1": np.asarray(W1, np.float32), "W2": np.asarray(W2, np.float32),
        "Wz": np.asarray(Wz, np.float32), "Wr": np.asarray(Wr, np.float32),
        "Wh": np.asarray(Wh, np.float32),
        "bz": np.asarray(bz, np.float32), "br": np.asarray(br, np.float32),
        "bh": np.asarray(bh, np.float32),
        "Whead": np.asarray(Whead, np.float32),
        "bhead": np.asarray(bhead, np.float32),
    }
    wxp = np.zeros((3, 16, 32, H), np.float32)
    for gi, W in enumerate((Wz, Wr, Wh)):
        Wx = np.asarray(W, np.float32)[0:F, :]
        for si in range(16):
            wxp[gi, si, 2 * si:2 * si + 2, :] = Wx
    common["WxP"] = wxp
    X = np.asarray(X, np.float32)
    in_maps = []
    for i in range(NCORES):
        xs = X[i * NB:(i + 1) * NB]                       # [NB, L, N, F]
        xn = np.zeros((N, NB, 64), np.float32)
        xn[:, :, :L * F] = xs.transpose(2, 0, 1, 3).reshape(N, NB, L * F)
        in_maps.append({"Xn": xn, **common})
    return in_maps


def run_on_hw(inputs, **kwargs):
    nc = _get_nc()
    in_maps = make_in_maps(**inputs)
    last_err = None
    for _attempt in range(3):
        try:
            res = run_bass_kernel_spmd(nc, in_maps,
                                       core_ids=list(range(NCORES)), **kwargs)
            break
        except Exception as e:  # transient NRT_EXEC_UNIT_UNRECOVERABLE
            last_err = e
            if "UNRECOVERABLE" not in str(e) and "UNAVAILABLE" not in str(e):
                raise
    else:
        raise last_err
    out = np.concatenate([res.results[i]["out"] for i in range(NCORES)], axis=0)
    return out, res


def kernel(**inputs) -> np.ndarray:
    out, _ = run_on_hw(inputs)
    return out
